# revision 26
# baseline (speedup 1.0000x reference)
"""Trainium2 Bass kernel for the cloth/obstacle nearest-face penalty criterion.

Problem (per batch b):
  centroids_cur = mean(obs_cur[faces], -2)              # [F,3]
  fidx  = argmin_f ||cloth_cur - centroids_cur||^2      # [Nc]
  plane = (unit normal(obs_nxt, faces), n.dot(centroid_nxt))  # [F,4]
  d     = cloth_nxt . n[fidx] - o[fidx]                 # [Nc]
  loss  = sum(relu(EPS - d)^3) / B * WEIGHT

Sharding (8 cores): data-parallel over B (2) x 4 shards of the cloth-node
axis (4096 nodes each); the obstacle mesh + faces are replicated per core.

Device algorithm per core:
  - gather face vertices with indirect DMA, build cent_ext [4, F] =
    (cx, cy, cz, |c|^2) and lhsT [4, 128] per node block = (-2px, -2py, -2pz, 1)
  - PE: v[n, f] = |c|^2 - 2 p.c  (argmin of v == argmin of squared distance)
  - ScalarE copies PSUM->SBUF; a custom single-pass DVE op (running-min scan +
    index-of-min accumulator) produces the argmin index per node directly
  - indirect-DMA gather of the plane payload by index, then the pointwise
    penalty; output is a [128, 1] vector of per-partition partial sums.
"""

import numpy as np

B, NC_FULL, NO, F = 2, 16384, 5000, 10000
NCORES = 8
NSHARD = 4  # node shards per batch
NC = NC_FULL // NSHARD  # 4096 nodes per core
NBLK = NC // 128  # 32 node blocks
FP = 10240  # padded face count (20 chunks of 512)
G = FP // 128  # 80 face groups of 128
GFULL = F // 128  # 78 full groups
TAIL = F - GFULL * 128  # 16
EPS = 0.004
WEIGHT = 5000.0
NORM_EPS = 1e-12
BIGC2 = 3.0e37  # pad-face squared norm; never wins the argmin
PSUM_COLS = 2048  # 4 PSUM banks per ScalarE copy
NGRP = FP // PSUM_COLS  # 5

_STATE = {}


def _register_argmin_op():
    """Register a custom DVE op: single-pass argmin along the free dim.

    r[k]    = min(s0_init, x[0..k])          (inclusive running min)
    body[k] = k if x[k] == r[k] else -FLT_MAX
    accum   = max(body)  ->  last index achieving the global min.
    """
    from concourse import dve_ops
    from concourse.dve_spec import (
        Spec, Src0, C0, MaxNeg, Idx, eq, select, AluOp, scan, lower,
    )
    from concourse.dve_uop import DveOpSpec

    for op in dve_ops.OPS:
        if op.name == "ARGMIN_SCAN_ANT":
            return op

    def _ref(in0, in1, s0, s1, imm2):
        r = np.minimum.accumulate(
            np.minimum(in0, np.asarray(s0).reshape(-1, 1)), axis=-1
        )
        body = np.where(in0 == r, np.arange(in0.shape[-1], dtype=np.float32),
                        np.float32(-3.4028235e38))
        return body, body.max(axis=-1, keepdims=True)

    r = scan(AluOp.MIN, Src0, init=C0)
    body = select(eq(Src0, r), Idx, MaxNeg)
    spec = Spec(body=body, accum=AluOp.MAX, accum_init=MaxNeg, reference=_ref)
    op = dve_ops.DveOp("ARGMIN_SCAN_ANT", spec, subdim=False, uops_sha={})
    dve_ops.OPS.append(op)
    dve_ops.CUSTOM_DVE_SPECS[op.name] = spec
    row = max(dve_ops._SUB_OPCODE_FOR_NAME.values()) + 1
    assert row < 0x20
    dve_ops._SUB_OPCODE_FOR_NAME[op.name] = row
    for ver in ("v3", "v4"):
        try:
            compiled = DveOpSpec(
                name=op.name, opcode=row, uops=lower(spec, ver=ver), rd1_en=False
            )
            op.uops_sha[ver] = compiled.sha(ver)
        except Exception:
            pass
    return op


def _build_kernel(debug_outs=False, reps=1):
    import concourse.bass as bass
    import concourse.bacc as bacc
    import concourse.mybir as mybir
    import concourse.tile as tile
    from concourse.bass import IndirectOffsetOnAxis

    argmin_op = _register_argmin_op()

    f32 = mybir.dt.float32
    i32 = mybir.dt.int32
    i16 = mybir.dt.int16
    Alu = mybir.AluOpType

    nc = bacc.Bacc(
        "TRN2",
        target_bir_lowering=False,
        debug=False,
        enable_asserts=False,
        num_devices=NCORES,
    )

    cloth_cur = nc.dram_tensor("cloth_cur", [NC, 3], f32, kind="ExternalInput").ap()
    cloth_nxt = nc.dram_tensor("cloth_nxt", [NC, 3], f32, kind="ExternalInput").ap()
    obs_cur = nc.dram_tensor("obs_cur", [NO, 3], f32, kind="ExternalInput").ap()
    obs_nxt = nc.dram_tensor("obs_nxt", [NO, 3], f32, kind="ExternalInput").ap()
    faces = nc.dram_tensor("faces", [F, 3], i32, kind="ExternalInput").ap()
    out_partial = nc.dram_tensor(
        "partial", [128, 1], f32, kind="ExternalOutput"
    ).ap()
    if debug_outs:
        dbg_fstar = nc.dram_tensor(
            "dbg_fstar", [128, NBLK], f32, kind="ExternalOutput").ap()
        dbg_payload = nc.dram_tensor(
            "dbg_payload", [128, NBLK, 4], f32, kind="ExternalOutput").ap()
        dbg_s0 = nc.dram_tensor(
            "dbg_s0", [128, FP], f32, kind="ExternalOutput").ap()
        dbg_cext = nc.dram_tensor(
            "dbg_cext", [4, FP], f32, kind="ExternalOutput").ap()
        dbg_plane = nc.dram_tensor(
            "dbg_plane", [F, 4], f32, kind="ExternalOutput").ap()

    with tile.TileContext(nc) as tc:
        with (
            tc.tile_pool(name="const", bufs=1) as cpool,
            tc.tile_pool(name="prep", bufs=1) as ppool,
            tc.tile_pool(name="s", bufs=2) as spool,
            tc.tile_pool(name="psum", bufs=2, space="PSUM") as pspool,
            tc.tile_pool(name="dram", bufs=1, space="DRAM") as dpool,
        ):
            # ---- scratch table: 32B rows holding cur(0:3) + nxt(3:6) -------
            tbl8 = dpool.tile([NO, 8], f32, tag="tbl8")
            nc.sync.dma_start(tbl8[:, 0:3], obs_cur)
            nc.sync.dma_start(tbl8[:, 3:6], obs_nxt)

            # ---- face index tiles [128, G] i32: idx[p, g] = faces[128g+p, k]
            idxf = []
            for k in range(3):
                it = ppool.tile([128, G], i32, tag=f"idx{k}")
                nc.vector.memset(it[:], 0)
                src = faces[: GFULL * 128, k : k + 1].rearrange(
                    "(g p) one -> p (g one)", p=128
                )
                nc.sync.dma_start(it[:, :GFULL], src)
                tl = faces[GFULL * 128 :, k : k + 1].rearrange(
                    "(g p) one -> p (g one)", p=TAIL
                )
                nc.sync.dma_start(it[:TAIL, GFULL : GFULL + 1], tl)
                idxf.append(it)

            # ---- gather vertices: 128 rows per indirect call ----------------
            vg8 = []
            for k in range(3):
                vt = ppool.tile([128, G, 8], f32, tag=f"vg8_{k}")
                for g in range(G):
                    nc.gpsimd.indirect_dma_start(
                        out=vt[:, g, :],
                        out_offset=None,
                        in_=tbl8[:],
                        in_offset=IndirectOffsetOnAxis(
                            ap=idxf[k][:, g : g + 1], axis=0
                        ),
                    )
                vg8.append(vt)

            cen_c = ppool.tile([128, G, 3], f32, tag="cen_c")
            cen_n = ppool.tile([128, G, 3], f32, tag="cen_n")
            e1 = ppool.tile([128, G, 3], f32, tag="e1")
            e2 = ppool.tile([128, G, 3], f32, tag="e2")
            nc.vector.tensor_tensor(
                cen_c[:], vg8[0][:, :, 0:3], vg8[1][:, :, 0:3], op=Alu.add
            )
            nc.vector.tensor_tensor(
                cen_c[:], cen_c[:], vg8[2][:, :, 0:3], op=Alu.add
            )
            nc.vector.tensor_tensor(
                cen_n[:], vg8[0][:, :, 3:6], vg8[1][:, :, 3:6], op=Alu.add
            )
            nc.vector.tensor_tensor(
                cen_n[:], cen_n[:], vg8[2][:, :, 3:6], op=Alu.add
            )
            nc.vector.tensor_tensor(
                e1[:], vg8[1][:, :, 3:6], vg8[0][:, :, 3:6], op=Alu.subtract
            )
            nc.vector.tensor_tensor(
                e2[:], vg8[2][:, :, 3:6], vg8[0][:, :, 3:6], op=Alu.subtract
            )

            # centroid (current) + |c|^2 -> cext [128, G, 4]
            cen = cen_c
            nc.vector.tensor_scalar_mul(cen[:], cen[:], 1.0 / 3.0)
            cext = ppool.tile([128, G, 4], f32, tag="cext")
            nc.vector.tensor_copy(cext[:, :, 0:3], cen[:])
            csq = ppool.tile([128, G, 3], f32, tag="csq")
            nc.vector.tensor_tensor(csq[:], cen[:], cen[:], op=Alu.mult)
            nc.vector.tensor_reduce(
                cext[:, :, 3:4], csq[:], axis=mybir.AxisListType.X, op=Alu.add
            )
            # pad faces (f >= F) get a huge |c|^2 so they never win the argmin
            fid_i = ppool.tile([128, G], i32, tag="fid_i")
            nc.gpsimd.iota(fid_i[:], pattern=[[128, G]], base=0,
                           channel_multiplier=1)
            fid_f = ppool.tile([128, G], f32, tag="fid_f")
            nc.vector.tensor_copy(fid_f[:], fid_i[:])
            nc.vector.tensor_scalar(
                fid_f[:], fid_f[:], float(F) - 0.5, BIGC2,
                op0=Alu.is_ge, op1=Alu.mult,
            )
            nc.vector.tensor_tensor(
                cext[:, :, 3], cext[:, :, 3], fid_f[:], op=Alu.add
            )

            # roundtrip through DRAM to get coord-major cent_ext [4, FP]
            cents_d = dpool.tile([G, 128, 4], f32, tag="cents_d")
            nc.sync.dma_start(
                cents_d.rearrange("g p c -> p g c"), cext[:]
            )
            cext_T = cpool.tile([4, FP], f32, tag="cext_T")
            nc.sync.dma_start(
                cext_T[:], cents_d.rearrange("g p c -> c (g p)")
            )

            # ---------------- plane table from obs_nxt ---------------------
            nrm = ppool.tile([128, G, 3], f32, tag="nrm")
            # cross product, one component at a time
            for c in range(3):
                a, b = (c + 1) % 3, (c + 2) % 3
                t0 = ppool.tile([128, G], f32, tag="xp0")
                nc.vector.tensor_tensor(
                    t0[:], e1[:, :, a], e2[:, :, b], op=Alu.mult
                )
                t1 = ppool.tile([128, G], f32, tag="xp1")
                nc.vector.tensor_tensor(
                    t1[:], e1[:, :, b], e2[:, :, a], op=Alu.mult
                )
                nc.vector.tensor_tensor(
                    nrm[:, :, c], t0[:], t1[:], op=Alu.subtract
                )
            nn = ppool.tile([128, G], f32, tag="nn")
            nsq = ppool.tile([128, G, 3], f32, tag="nsq")
            nc.vector.tensor_tensor(nsq[:], nrm[:], nrm[:], op=Alu.mult)
            nc.vector.tensor_reduce(
                nn[:], nsq[:], axis=mybir.AxisListType.X, op=Alu.add
            )
            nc.vector.tensor_scalar_max(nn[:], nn[:], NORM_EPS)
            sqv = ppool.tile([128, G], f32, tag="sqv")
            nc.scalar.activation(
                sqv[:], nn[:], mybir.ActivationFunctionType.Sqrt
            )
            rsq = ppool.tile([128, G], f32, tag="rsq")
            nc.vector.reciprocal(rsq[:], sqv[:])
            # one Newton step: r *= 1.5 - 0.5 * nn * r^2
            r2 = ppool.tile([128, G], f32, tag="r2")
            nc.vector.tensor_tensor(r2[:], rsq[:], rsq[:], op=Alu.mult)
            nc.vector.tensor_tensor(r2[:], r2[:], nn[:], op=Alu.mult)
            nc.vector.tensor_scalar(
                r2[:], r2[:], -0.5, 1.5, op0=Alu.mult, op1=Alu.add
            )
            nc.vector.tensor_tensor(rsq[:], rsq[:], r2[:], op=Alu.mult)

            plane = ppool.tile([128, G, 4], f32, tag="plane")
            for c in range(3):
                nc.vector.tensor_tensor(
                    plane[:, :, c], nrm[:, :, c], rsq[:], op=Alu.mult
                )
            # o = n_hat . centroid_nxt
            nc.vector.tensor_scalar_mul(cen_n[:], cen_n[:], 1.0 / 3.0)
            od = ppool.tile([128, G, 3], f32, tag="od")
            nc.vector.tensor_tensor(od[:], cen_n[:], plane[:, :, 0:3], op=Alu.mult)
            nc.vector.tensor_reduce(
                plane[:, :, 3:4], od[:], axis=mybir.AxisListType.X, op=Alu.add
            )
            plane_d = dpool.tile([G, 128, 4], f32, tag="plane_d")
            nc.sync.dma_start(plane_d.rearrange("g p c -> p g c"), plane[:])
            plane_flat = plane_d.rearrange("g p c -> (g p) c")

            # ---------------- lhsT: points_ext [4, NBLK, 128] --------------
            ptsT = cpool.tile([4, NBLK, 128], f32, tag="ptsT")
            # row 3 stays 1.0; rows 0..2 <- -2 * cloth_cur (coord-major DMA)
            nc.vector.memset(ptsT[:], 1.0)
            nc.sync.dma_start(
                ptsT[0:3, :, :],
                cloth_cur.rearrange("(b p) c -> c b p", p=128),
            )
            nc.vector.tensor_scalar_mul(ptsT[0:3, :, :], ptsT[0:3, :, :], -2.0)

            # ---------------- main loop: distances + argmin -----------------
            fstar = cpool.tile([128, NBLK], f32, tag="fstar")
            fidx = cpool.tile([128, NBLK], i32, tag="fidx")
            payload = cpool.tile([128, NBLK, 4], f32, tag="payload")
            for b0 in range(NBLK * reps):
                b = b0 % NBLK
                s_tile = spool.tile([128, FP], f32, tag="s")
                for g in range(NGRP):
                    ps = pspool.tile([128, PSUM_COLS], f32, tag="ps")
                    for j in range(PSUM_COLS // 512):
                        col = g * PSUM_COLS + j * 512
                        nc.tensor.matmul(
                            ps[:, j * 512 : (j + 1) * 512],
                            lhsT=ptsT[:, b, :],
                            rhs=cext_T[:, col : col + 512],
                            start=True,
                            stop=True,
                        )
                    nc.scalar.copy(
                        s_tile[:, g * PSUM_COLS : (g + 1) * PSUM_COLS], ps[:]
                    )
                if debug_outs and b == 0:
                    nc.sync.dma_start(dbg_s0, s_tile[:])
                nc.vector._custom_dve(
                    argmin_op,
                    out=s_tile[:],
                    in0=s_tile[:],
                    s0=3.0e38,
                    accum_out=fstar[:, b : b + 1],
                )
                # payload gather overlaps later blocks' compute
                nc.vector.tensor_copy(fidx[:, b : b + 1], fstar[:, b : b + 1])
                nc.gpsimd.indirect_dma_start(
                    out=payload[:, b, :],
                    out_offset=None,
                    in_=plane_flat,
                    in_offset=IndirectOffsetOnAxis(
                        ap=fidx[:, b : b + 1], axis=0
                    ),
                )
            clothn = cpool.tile([128, NBLK, 3], f32, tag="clothn")
            nc.sync.dma_start(
                clothn[:], cloth_nxt.rearrange("(b p) c -> p b c", p=128)
            )
            dtmp = cpool.tile([128, NBLK, 3], f32, tag="dtmp")
            nc.vector.tensor_tensor(
                dtmp[:], clothn[:], payload[:, :, 0:3], op=Alu.mult
            )
            dist = cpool.tile([128, NBLK], f32, tag="dist")
            nc.vector.tensor_reduce(
                dist[:], dtmp[:], axis=mybir.AxisListType.X, op=Alu.add
            )
            nc.vector.tensor_tensor(
                dist[:], dist[:], payload[:, :, 3], op=Alu.subtract
            )
            # t = relu(EPS - d); partial = sum(t^3)
            nc.vector.tensor_scalar(
                dist[:], dist[:], -1.0, EPS, op0=Alu.mult, op1=Alu.add
            )
            nc.vector.tensor_scalar_max(dist[:], dist[:], 0.0)
            sq = cpool.tile([128, NBLK], f32, tag="sq")
            nc.vector.tensor_tensor(sq[:], dist[:], dist[:], op=Alu.mult)
            psum_out = cpool.tile([128, 1], f32, tag="psum_out")
            nc.vector.scalar_tensor_tensor(
                out=sq[:],
                in0=sq[:],
                scalar=1.0,
                in1=dist[:],
                op0=Alu.mult,
                op1=Alu.mult,
                accum_out=psum_out[:],
            )
            nc.sync.dma_start(out_partial, psum_out[:])
            if debug_outs:
                nc.sync.dma_start(dbg_fstar, fstar[:])
                nc.sync.dma_start(dbg_payload, payload[:])
                nc.sync.dma_start(dbg_cext, cext_T[:])
                nc.sync.dma_start(
                    dbg_plane, plane_flat[:F, :]
                )

    nc.compile()
    return nc


def _get_nc():
    if "nc" not in _STATE:
        _STATE["nc"] = _build_kernel()
    return _STATE["nc"]


def kernel(cloth_pos, cloth_pred_pos, obstacle_pos, obstacle_next_pos,
           obstacle_faces):
    from concourse.bass_utils import run_bass_kernel_spmd

    nc = _get_nc()
    in_maps = []
    for core in range(NCORES):
        b, sh = divmod(core, NSHARD)
        sl = slice(sh * NC, (sh + 1) * NC)
        in_maps.append({
            "cloth_cur": np.ascontiguousarray(cloth_pos[b, sl], dtype=np.float32),
            "cloth_nxt": np.ascontiguousarray(
                cloth_pred_pos[b, sl], dtype=np.float32
            ),
            "obs_cur": np.ascontiguousarray(obstacle_pos[b], dtype=np.float32),
            "obs_nxt": np.ascontiguousarray(
                obstacle_next_pos[b], dtype=np.float32
            ),
            "faces": np.ascontiguousarray(obstacle_faces, dtype=np.int32),
        })
    res = run_bass_kernel_spmd(nc, in_maps, core_ids=list(range(NCORES)))
    total = sum(
        float(r["partial"].sum(dtype=np.float64)) for r in res.results
    )
    return np.float32(total / B * WEIGHT)
